# revision 9
# baseline (speedup 1.0000x reference)
"""AdaptiveCenterLoss on 8 TRN2 NeuronCores.

loss = sum((data - cen[labels])**2) / BATCH

Data-parallel over batch: each core handles 8192 rows, gathers its
center rows from a replicated `cen` table via indirect DMA (the
embedding lookup), computes (data-center)^2, and DMAs per-partition
partials out; the host sums partials across partitions/cores.

The kernel is HBM-bound, and the 2e-2 rel-err budget dwarfs bf16
rounding noise (~3e-4 on this sum), so the host downcasts data/cen to
bf16 before upload — halving both the contiguous data stream and the
gather traffic.

Host prep: each core's 8192 rows are sorted by label so the gather's
descriptors walk the center table near-sequentially (DRAM locality);
the row sum is permutation-invariant. The first K_LIST[0] rows per
partition get their centers gathered on the host (cen0) so tile 0 is
two direct DMAs — compute starts ~5us before the first on-device
gather can land (labels DMA -> GPSIMD descriptor gen -> SWDGE).

Every tile gets its own SBUF buffer (64 rows/partition x 256 x bf16 x
2 tensors = 64KB/partition of the 208KB budget) so no DMA ever waits
on buffer recycling: all 9 data-tile DMAs issue the moment the NEFF
starts, and gathers issue as fast as GPSIMD generates descriptors.
Per tile, DVE computes the diff; the square+row-sum is split
ACT_FRAC/(1-ACT_FRAC) between ACT (Square w/ accumulate) and DVE
(affine_mul_reduce) so neither engine paces the stream.
"""

import os

import numpy as np

BATCH = 65536
DIM = 256
NUM_CLASSES = 100000
N_CORES = 8
B_CORE = BATCH // N_CORES  # 8192

P = 128               # SBUF partitions
R = B_CORE // P       # rows per partition (64)

# Tile 0 (host-gathered centers) first; small early tiles so the
# gather pipeline primes quickly.
_klist_env = os.environ.get("ACL_KLIST", "2,4,8,8,8,8,8,8,4,4,2")
K_LIST = [int(x) for x in _klist_env.split(",")]
assert sum(K_LIST) == R, K_LIST
NT = len(K_LIST)
DT = os.environ.get("ACL_DTYPE", "bf16")
ACT_FRAC = float(os.environ.get("ACL_ACT_FRAC", "0.69"))
TAIL_CHUNKS = int(os.environ.get("ACL_TAILCHUNKS", "1"))
FASTLAB = os.environ.get("ACL_FASTLAB", "0") == "1"
SORT = os.environ.get("ACL_SORT", "1") == "1"
HOSTG0 = os.environ.get("ACL_HOSTG0", "1") == "1"
DATADT = os.environ.get("ACL_DATADT", "f8")  # data stream dtype: f8 | same
WARMG = os.environ.get("ACL_WARMG", "1") == "1"

_cached = {}


def _build_graph():
    from concourse import bass, bacc, mybir, tile

    nc = bacc.Bacc(
        "TRN2",
        target_bir_lowering=False,
        debug=False,
        num_devices=N_CORES,
    )
    f32 = mybir.dt.float32
    i32 = mybir.dt.int32
    vdt = mybir.dt.bfloat16 if DT == "bf16" else f32
    ddt = mybir.dt.float8e4 if DATADT == "f8" else vdt

    n_cols = 2 * (NT - 1 + TAIL_CHUNKS)  # (ACT, DVE) partial per chunk

    data_t = nc.dram_tensor("data", [P, R * DIM], ddt, kind="ExternalInput")
    lab_shape = [R, P] if FASTLAB else [P, R]
    lab_t = nc.dram_tensor("labels", lab_shape, i32, kind="ExternalInput")
    cen_t = nc.dram_tensor("cen", [NUM_CLASSES, DIM], vdt, kind="ExternalInput")
    if HOSTG0:
        cen0_t = nc.dram_tensor(
            "cen0", [P, K_LIST[0] * DIM], vdt, kind="ExternalInput"
        )
    out_t = nc.dram_tensor("out", [P, n_cols], f32, kind="ExternalOutput")

    with tile.TileContext(nc) as tc:
        with tc.tile_pool(name="persist", bufs=1) as persist:
            labs = persist.tile([P, R], i32)
            if FASTLAB:
                # Labels arrive transposed [64, 128]: 64 512B descriptors
                # instead of 128 256B ones; 8 DVE 32x32 block transposes
                # restore [128, 64], low-column blocks first.
                labs64 = persist.tile([R, P], i32)
                nc.sync.dma_start(out=labs64[:], in_=lab_t.ap()[:])
                for b in range(R // 32):
                    for a in range(P // 32):
                        nc.vector.transpose(
                            out=labs[32 * a:32 * a + 32, 32 * b:32 * b + 32],
                            in_=labs64[32 * b:32 * b + 32, 32 * a:32 * a + 32],
                        )
            else:
                nc.sync.dma_start(out=labs[:], in_=lab_t.ap()[:])

            # Dedicated buffers per tile: DMAs never wait on recycling.
            ctrs = [persist.tile([P, k * DIM], vdt, name=f"ctr{t}")
                    for t, k in enumerate(K_LIST)]
            dats = [persist.tile([P, k * DIM], ddt, name=f"dat{t}")
                    for t, k in enumerate(K_LIST)]
            parts = persist.tile([P, n_cols], f32)

            # All data-tile loads (and tile 0's direct center load) are
            # issued up front with no dependencies.
            off = 0
            for t, k in enumerate(K_LIST):
                nc.sync.dma_start(
                    out=dats[t][:],
                    in_=data_t.ap()[:, off * DIM:(off + k) * DIM],
                )
                off += k
            if HOSTG0:
                nc.sync.dma_start(out=ctrs[0][:], in_=cen0_t.ap()[:])

            if WARMG:
                # Dummy 2-row gather with constant offsets: pays the
                # SWDGE queue warmup latency before the real gathers
                # (which must wait for the labels DMA) need it.
                woff = persist.tile([P, 2], i32)
                wdst = persist.tile([P, 2 * DIM], vdt)
                nc.gpsimd.memset(woff[:], 0)
                nc.gpsimd.indirect_dma_start(
                    out=wdst[:],
                    out_offset=None,
                    in_=cen_t.ap()[:],
                    in_offset=bass.IndirectOffsetOnAxis(ap=woff[:], axis=0),
                )

            # Gathers: descriptor gen on GPSIMD (serial), paced only by
            # the labels DMA.
            off = 0
            for t, k in enumerate(K_LIST):
                if t > 0 or not HOSTG0:
                    nc.gpsimd.indirect_dma_start(
                        out=ctrs[t][:],
                        out_offset=None,
                        in_=cen_t.ap()[:],
                        in_offset=bass.IndirectOffsetOnAxis(
                            ap=labs[:, off:off + k], axis=0
                        ),
                    )
                off += k

            col = 0
            for t, k in enumerate(K_LIST):
                dat, ctr = dats[t], ctrs[t]
                last = t == len(K_LIST) - 1
                chunks = TAIL_CHUNKS if last and k % TAIL_CHUNKS == 0 else 1
                cw = k * DIM // chunks
                for c in range(chunks):
                    lo = c * cw
                    ca = lo + min(cw, max(32, int(cw * ACT_FRAC) // 32 * 32))
                    hi = lo + cw
                    nc.vector.tensor_tensor(
                        out=ctr[:, lo:hi], in0=dat[:, lo:hi], in1=ctr[:, lo:hi],
                        op=mybir.AluOpType.subtract,
                    )
                    # Squares' dead outputs overwrite the (consumed) data.
                    nc.scalar.activation(
                        dat[:, lo:ca], ctr[:, lo:ca],
                        mybir.ActivationFunctionType.Square,
                        accum_out=parts[:, col:col + 1],
                    )
                    if ca < hi:
                        nc.vector.affine_mul_reduce(
                            out=dat[:, ca:hi],
                            accum_out=parts[:, col + 1:col + 2],
                            in0=ctr[:, ca:hi], in1=ctr[:, ca:hi],
                            scale=1.0, bias=0.0,
                        )
                    else:
                        nc.vector.memset(parts[:, col + 1:col + 2], 0.0)
                    col += 2

            nc.sync.dma_start(out=out_t.ap()[:], in_=parts[:])

    nc.compile()
    return nc


def _get_graph():
    if "nc" not in _cached:
        _cached["nc"] = _build_graph()
    return _cached["nc"]


def _val_dtype():
    if DT == "bf16":
        import ml_dtypes

        return ml_dtypes.bfloat16
    return np.float32


def _data_dtype():
    if DATADT == "f8":
        import ml_dtypes

        return ml_dtypes.float8_e4m3
    return _val_dtype()


def _make_in_maps(data, cen, labels):
    vdt = _val_dtype()
    data = np.ascontiguousarray(np.asarray(data)).astype(_data_dtype())
    cen = np.ascontiguousarray(np.asarray(cen)).astype(vdt)
    labels = np.asarray(labels).astype(np.int32)
    in_maps = []
    for c in range(N_CORES):
        sl = slice(c * B_CORE, (c + 1) * B_CORE)
        dat_c = data[sl]
        lab_c = labels[sl]
        if SORT:
            # Sort rows by label: the gather descriptors then walk cen
            # near-sequentially (DRAM page locality). Sum is invariant.
            order = np.argsort(lab_c)
            dat_c = dat_c[order]
            lab_c = lab_c[order]
        lab2d = lab_c.reshape(P, R)
        if FASTLAB:
            lab_up = np.ascontiguousarray(lab2d.T)
        else:
            lab_up = np.ascontiguousarray(lab2d)
        m = {
            "data": dat_c.reshape(P, R * DIM),
            "labels": lab_up,
            "cen": cen,
        }
        if HOSTG0:
            k0 = K_LIST[0]
            m["cen0"] = cen[lab2d[:, :k0].ravel()].reshape(P, k0 * DIM)
        in_maps.append(m)
    return in_maps


def _run(data, cen, labels, trace=False):
    import time

    from concourse.bass_utils import run_bass_kernel_spmd

    nc = _get_graph()
    in_maps = _make_in_maps(data, cen, labels)
    last_err = None
    for attempt in range(4):
        try:
            res = run_bass_kernel_spmd(
                nc, in_maps, core_ids=list(range(N_CORES)), trace=trace
            )
        except Exception as e:  # transient NRT device flakes
            last_err = e
            time.sleep(2.0)
            continue
        total = float(
            np.sum(
                [res.results[i]["out"].astype(np.float64) for i in range(N_CORES)]
            )
        )
        if np.isfinite(total):  # rare cold-start flake: garbage gather
            return np.float32(total / BATCH), res
    if last_err is not None:
        raise last_err
    return np.float32(total / BATCH), res


def kernel(data, cen, labels):
    out, _ = _run(data, cen, labels)
    return out


# revision 11
# speedup vs baseline: 1.1043x; 1.1043x over previous
"""AdaptiveCenterLoss on 8 TRN2 NeuronCores.

loss = sum((data - cen[labels])**2) / BATCH

Data-parallel over batch: each core handles 8192 rows, gathers its
center rows from a replicated `cen` table via indirect DMA (the
embedding lookup), computes (data-center)^2, and DMAs per-partition
partials out; the host sums partials across partitions/cores.

The kernel is HBM-bound, and the 2e-2 rel-err budget dwarfs bf16
rounding noise (~3e-4 on this sum), so the host downcasts data/cen to
bf16 before upload — halving both the contiguous data stream and the
gather traffic.

Host prep: each core's 8192 rows are sorted by label so the gather's
descriptors walk the center table near-sequentially (DRAM locality);
the row sum is permutation-invariant. The first K_LIST[0] rows per
partition get their centers gathered on the host (cen0) so tile 0 is
two direct DMAs — compute starts ~5us before the first on-device
gather can land (labels DMA -> GPSIMD descriptor gen -> SWDGE).

Every tile gets its own SBUF buffer (64 rows/partition x 256 x bf16 x
2 tensors = 64KB/partition of the 208KB budget) so no DMA ever waits
on buffer recycling: all 9 data-tile DMAs issue the moment the NEFF
starts, and gathers issue as fast as GPSIMD generates descriptors.
Per tile, DVE computes the diff; the square+row-sum is split
ACT_FRAC/(1-ACT_FRAC) between ACT (Square w/ accumulate) and DVE
(affine_mul_reduce) so neither engine paces the stream.
"""

import os

import numpy as np

BATCH = 65536
DIM = 256
NUM_CLASSES = 100000
N_CORES = 8
B_CORE = BATCH // N_CORES  # 8192

P = 128               # SBUF partitions
R = B_CORE // P       # rows per partition (64)

# Tile 0 (host-gathered centers) first; small early tiles so the
# gather pipeline primes quickly.
_klist_env = os.environ.get("ACL_KLIST", "2,4,8,8,8,8,8,8,4,4,2")
K_LIST = [int(x) for x in _klist_env.split(",")]
assert sum(K_LIST) == R, K_LIST
NT = len(K_LIST)
DT = os.environ.get("ACL_DTYPE", "bf16")
ACT_FRAC = float(os.environ.get("ACL_ACT_FRAC", "0.69"))
TAIL_CHUNKS = int(os.environ.get("ACL_TAILCHUNKS", "1"))
FASTLAB = os.environ.get("ACL_FASTLAB", "0") == "1"
SORT = os.environ.get("ACL_SORT", "1") == "1"
HOSTG0 = os.environ.get("ACL_HOSTG0", "1") == "1"
IMPL = os.environ.get("ACL_IMPL", "tile")
DATADT = os.environ.get("ACL_DATADT", "same")  # f8 halves bytes but DVE sub drops to 1 elem/cycle
WARMG = os.environ.get("ACL_WARMG", "1") == "1"

_cached = {}


def _build_graph():
    from concourse import bass, bacc, mybir, tile

    nc = bacc.Bacc(
        "TRN2",
        target_bir_lowering=False,
        debug=False,
        num_devices=N_CORES,
    )
    f32 = mybir.dt.float32
    i32 = mybir.dt.int32
    vdt = mybir.dt.bfloat16 if DT == "bf16" else f32
    ddt = mybir.dt.float8e4 if DATADT == "f8" else vdt

    n_cols = 2 * (NT - 1 + TAIL_CHUNKS)  # (ACT, DVE) partial per chunk

    data_t = nc.dram_tensor("data", [P, R * DIM], ddt, kind="ExternalInput")
    lab_shape = [R, P] if FASTLAB else [P, R]
    lab_t = nc.dram_tensor("labels", lab_shape, i32, kind="ExternalInput")
    cen_t = nc.dram_tensor("cen", [NUM_CLASSES, DIM], vdt, kind="ExternalInput")
    if HOSTG0:
        cen0_t = nc.dram_tensor(
            "cen0", [P, K_LIST[0] * DIM], vdt, kind="ExternalInput"
        )
    out_t = nc.dram_tensor("out", [P, n_cols], f32, kind="ExternalOutput")

    with tile.TileContext(nc) as tc:
        with tc.tile_pool(name="persist", bufs=1) as persist:
            labs = persist.tile([P, R], i32)
            if FASTLAB:
                # Labels arrive transposed [64, 128]: 64 512B descriptors
                # instead of 128 256B ones; 8 DVE 32x32 block transposes
                # restore [128, 64], low-column blocks first.
                labs64 = persist.tile([R, P], i32)
                nc.sync.dma_start(out=labs64[:], in_=lab_t.ap()[:])
                for b in range(R // 32):
                    for a in range(P // 32):
                        nc.vector.transpose(
                            out=labs[32 * a:32 * a + 32, 32 * b:32 * b + 32],
                            in_=labs64[32 * b:32 * b + 32, 32 * a:32 * a + 32],
                        )
            else:
                nc.sync.dma_start(out=labs[:], in_=lab_t.ap()[:])

            # Dedicated buffers per tile: DMAs never wait on recycling.
            ctrs = [persist.tile([P, k * DIM], vdt, name=f"ctr{t}")
                    for t, k in enumerate(K_LIST)]
            dats = [persist.tile([P, k * DIM], ddt, name=f"dat{t}")
                    for t, k in enumerate(K_LIST)]
            parts = persist.tile([P, n_cols], f32)

            # All data-tile loads (and tile 0's direct center load) are
            # issued up front with no dependencies.
            off = 0
            for t, k in enumerate(K_LIST):
                nc.sync.dma_start(
                    out=dats[t][:],
                    in_=data_t.ap()[:, off * DIM:(off + k) * DIM],
                )
                off += k
            if HOSTG0:
                nc.sync.dma_start(out=ctrs[0][:], in_=cen0_t.ap()[:])

            if WARMG:
                # Dummy 2-row gather with constant offsets: pays the
                # SWDGE queue warmup latency before the real gathers
                # (which must wait for the labels DMA) need it.
                woff = persist.tile([P, 2], i32)
                wdst = persist.tile([P, 2 * DIM], vdt)
                nc.gpsimd.memset(woff[:], 0)
                nc.gpsimd.indirect_dma_start(
                    out=wdst[:],
                    out_offset=None,
                    in_=cen_t.ap()[:],
                    in_offset=bass.IndirectOffsetOnAxis(ap=woff[:], axis=0),
                )

            # Gathers: descriptor gen on GPSIMD (serial), paced only by
            # the labels DMA.
            off = 0
            for t, k in enumerate(K_LIST):
                if t > 0 or not HOSTG0:
                    nc.gpsimd.indirect_dma_start(
                        out=ctrs[t][:],
                        out_offset=None,
                        in_=cen_t.ap()[:],
                        in_offset=bass.IndirectOffsetOnAxis(
                            ap=labs[:, off:off + k], axis=0
                        ),
                    )
                off += k

            col = 0
            for t, k in enumerate(K_LIST):
                dat, ctr = dats[t], ctrs[t]
                last = t == len(K_LIST) - 1
                chunks = TAIL_CHUNKS if last and k % TAIL_CHUNKS == 0 else 1
                cw = k * DIM // chunks
                for c in range(chunks):
                    lo = c * cw
                    ca = lo + min(cw, max(32, int(cw * ACT_FRAC) // 32 * 32))
                    hi = lo + cw
                    nc.vector.tensor_tensor(
                        out=ctr[:, lo:hi], in0=dat[:, lo:hi], in1=ctr[:, lo:hi],
                        op=mybir.AluOpType.subtract,
                    )
                    # Squares' dead outputs overwrite the (consumed) data.
                    nc.scalar.activation(
                        dat[:, lo:ca], ctr[:, lo:ca],
                        mybir.ActivationFunctionType.Square,
                        accum_out=parts[:, col:col + 1],
                    )
                    if ca < hi:
                        nc.vector.affine_mul_reduce(
                            out=dat[:, ca:hi],
                            accum_out=parts[:, col + 1:col + 2],
                            in0=ctr[:, ca:hi], in1=ctr[:, ca:hi],
                            scale=1.0, bias=0.0,
                        )
                    else:
                        nc.vector.memset(parts[:, col + 1:col + 2], 0.0)
                    col += 2

            nc.sync.dma_start(out=out_t.ap()[:], in_=parts[:])

    nc.compile()
    return nc




def _build_graph_raw():
    """Raw-engine pipeline: same dataflow as the tile impl, but no
    TileContext prologue/epilogue barriers and explicit per-tile
    semaphores. Engine programs are in-order, so cumulative counting
    sems (sub/act/amr) are safe."""
    from contextlib import ExitStack

    from concourse import bass, bacc, mybir

    nc = bacc.Bacc(
        "TRN2",
        target_bir_lowering=False,
        debug=False,
        num_devices=N_CORES,
    )
    f32 = mybir.dt.float32
    i32 = mybir.dt.int32
    vdt = mybir.dt.bfloat16 if DT == "bf16" else f32
    ddt = mybir.dt.float8e4 if DATADT == "f8" else vdt

    chunks_of = [
        TAIL_CHUNKS if t == NT - 1 and k % TAIL_CHUNKS == 0 else 1
        for t, k in enumerate(K_LIST)
    ]
    n_chunks = sum(chunks_of)
    n_cols = 2 * n_chunks

    data_t = nc.dram_tensor("data", [P, R * DIM], ddt, kind="ExternalInput")
    lab_t = nc.dram_tensor("labels", [P, R], i32, kind="ExternalInput")
    cen_t = nc.dram_tensor("cen", [NUM_CLASSES, DIM], vdt, kind="ExternalInput")
    if HOSTG0:
        cen0_t = nc.dram_tensor(
            "cen0", [P, K_LIST[0] * DIM], vdt, kind="ExternalInput"
        )
    out_t = nc.dram_tensor("out", [P, n_cols], f32, kind="ExternalOutput")

    labs = nc.alloc_sbuf_tensor("labs", [P, R], i32)
    parts = nc.alloc_sbuf_tensor("parts", [P, n_cols], f32)
    bias = nc.alloc_sbuf_tensor("bias", [P, 1], f32)
    ctrs = [
        nc.alloc_sbuf_tensor(f"ctr{t}", [P, k * DIM], vdt)
        for t, k in enumerate(K_LIST)
    ]
    dats = [
        nc.alloc_sbuf_tensor(f"dat{t}", [P, k * DIM], ddt)
        for t, k in enumerate(K_LIST)
    ]
    if WARMG:
        woff = nc.alloc_sbuf_tensor("woff", [P, 2], i32)
        wdst = nc.alloc_sbuf_tensor("wdst", [P, 2 * DIM], vdt)

    with ExitStack() as es:
        block = es.enter_context(nc.Block(no_gpsimd_drain=True))
        lab_sem = es.enter_context(nc.semaphore("lab_sem"))
        out_sem = es.enter_context(nc.semaphore("out_sem"))
        sub_sem = es.enter_context(nc.semaphore("sub_sem"))
        act_sem = es.enter_context(nc.semaphore("act_sem"))
        amr_sem = es.enter_context(nc.semaphore("amr_sem"))
        dat_sems = [
            es.enter_context(nc.semaphore(f"dat_sem{t}")) for t in range(NT)
        ]
        ctr_sems = [
            es.enter_context(nc.semaphore(f"ctr_sem{t}")) for t in range(NT)
        ]

        def col_plan():
            col = 0
            for t, k in enumerate(K_LIST):
                cw = k * DIM // chunks_of[t]
                for c in range(chunks_of[t]):
                    lo = c * cw
                    ca = lo + min(cw, max(32, int(cw * ACT_FRAC) // 32 * 32))
                    yield t, lo, ca, lo + cw, col
                    col += 2

        @block.sync
        def _(sync: bass.BassEngine):
            sync.dma_start(out=labs.ap()[:], in_=lab_t.ap()[:]).then_inc(
                lab_sem, 16
            )
            if HOSTG0:
                sync.dma_start(out=ctrs[0].ap()[:], in_=cen0_t.ap()[:]).then_inc(
                    ctr_sems[0], 16
                )
            off = 0
            for t, k in enumerate(K_LIST):
                sync.dma_start(
                    out=dats[t].ap()[:],
                    in_=data_t.ap()[:, off * DIM:(off + k) * DIM],
                ).then_inc(dat_sems[t], 16)
                off += k
            sync.wait_ge(act_sem, n_chunks)
            sync.wait_ge(amr_sem, n_chunks)
            sync.dma_start(out=out_t.ap()[:], in_=parts.ap()[:]).then_inc(
                out_sem, 16
            )
            sync.wait_ge(out_sem, 16)

        @block.gpsimd
        def _(gpsimd: bass.BassEngine):
            if WARMG:
                gpsimd.memset(woff.ap()[:], 0)
                gpsimd.indirect_dma_start(
                    out=wdst.ap()[:],
                    out_offset=None,
                    in_=cen_t.ap()[:],
                    in_offset=bass.IndirectOffsetOnAxis(ap=woff.ap()[:], axis=0),
                )
            gpsimd.wait_ge(lab_sem, 16)
            off = 0
            for t, k in enumerate(K_LIST):
                if t > 0 or not HOSTG0:
                    gpsimd.indirect_dma_start(
                        out=ctrs[t].ap()[:],
                        out_offset=None,
                        in_=cen_t.ap()[:],
                        in_offset=bass.IndirectOffsetOnAxis(
                            ap=labs.ap()[:, off:off + k], axis=0
                        ),
                    ).then_inc(ctr_sems[t], 16)
                off += k

        @block.vector
        def _(vector: bass.BassEngine):
            vector.memset(bias.ap()[:], 0.0)
            seen = set()
            for t, lo, ca, hi, col in col_plan():
                if t not in seen:
                    seen.add(t)
                    vector.wait_ge(dat_sems[t], 16)
                    vector.wait_ge(ctr_sems[t], 16)
                vector.tensor_tensor(
                    out=ctrs[t].ap()[:, lo:hi],
                    in0=dats[t].ap()[:, lo:hi],
                    in1=ctrs[t].ap()[:, lo:hi],
                    op=mybir.AluOpType.subtract,
                ).then_inc(sub_sem, 1)
                if ca < hi:
                    vector.affine_mul_reduce(
                        out=dats[t].ap()[:, ca:hi],
                        accum_out=parts.ap()[:, col + 1:col + 2],
                        in0=ctrs[t].ap()[:, ca:hi],
                        in1=ctrs[t].ap()[:, ca:hi],
                        scale=1.0,
                        bias=0.0,
                    ).then_inc(amr_sem, 1)
                else:
                    vector.memset(parts.ap()[:, col + 1:col + 2], 0.0).then_inc(
                        amr_sem, 1
                    )

        @block.scalar
        def _(scalar: bass.BassEngine):
            i = 0
            for t, lo, ca, hi, col in col_plan():
                scalar.wait_ge(sub_sem, i + 1)
                scalar.activation(
                    dats[t].ap()[:, lo:ca],
                    ctrs[t].ap()[:, lo:ca],
                    mybir.ActivationFunctionType.Square,
                    bias=bias.ap()[:, :1],
                    accum_out=parts.ap()[:, col:col + 1],
                ).then_inc(act_sem, 1)
                i += 1

    nc.compile()
    return nc


def _get_graph():
    if "nc" not in _cached:
        _cached["nc"] = (
            _build_graph_raw() if IMPL == "raw" else _build_graph()
        )
    return _cached["nc"]


def _val_dtype():
    if DT == "bf16":
        import ml_dtypes

        return ml_dtypes.bfloat16
    return np.float32


def _data_dtype():
    if DATADT == "f8":
        import ml_dtypes

        return ml_dtypes.float8_e4m3
    return _val_dtype()


def _make_in_maps(data, cen, labels):
    vdt = _val_dtype()
    data = np.ascontiguousarray(np.asarray(data)).astype(_data_dtype())
    cen = np.ascontiguousarray(np.asarray(cen)).astype(vdt)
    labels = np.asarray(labels).astype(np.int32)
    in_maps = []
    for c in range(N_CORES):
        sl = slice(c * B_CORE, (c + 1) * B_CORE)
        dat_c = data[sl]
        lab_c = labels[sl]
        if SORT:
            # Sort rows by label: the gather descriptors then walk cen
            # near-sequentially (DRAM page locality). Sum is invariant.
            order = np.argsort(lab_c)
            dat_c = dat_c[order]
            lab_c = lab_c[order]
        lab2d = lab_c.reshape(P, R)
        if FASTLAB:
            lab_up = np.ascontiguousarray(lab2d.T)
        else:
            lab_up = np.ascontiguousarray(lab2d)
        m = {
            "data": dat_c.reshape(P, R * DIM),
            "labels": lab_up,
            "cen": cen,
        }
        if HOSTG0:
            k0 = K_LIST[0]
            m["cen0"] = cen[lab2d[:, :k0].ravel()].reshape(P, k0 * DIM)
        in_maps.append(m)
    return in_maps


def _run(data, cen, labels, trace=False):
    import time

    from concourse.bass_utils import run_bass_kernel_spmd

    nc = _get_graph()
    in_maps = _make_in_maps(data, cen, labels)
    last_err = None
    for attempt in range(4):
        try:
            res = run_bass_kernel_spmd(
                nc, in_maps, core_ids=list(range(N_CORES)), trace=trace
            )
        except Exception as e:  # transient NRT device flakes
            last_err = e
            time.sleep(2.0)
            continue
        total = float(
            np.sum(
                [res.results[i]["out"].astype(np.float64) for i in range(N_CORES)]
            )
        )
        if np.isfinite(total):  # rare cold-start flake: garbage gather
            return np.float32(total / BATCH), res
    if last_err is not None:
        raise last_err
    return np.float32(total / BATCH), res


def kernel(data, cen, labels):
    out, _ = _run(data, cen, labels)
    return out


# revision 15
# speedup vs baseline: 1.1866x; 1.0745x over previous
"""AdaptiveCenterLoss on 8 TRN2 NeuronCores.

loss = sum((data - cen[labels])**2) / BATCH

Data-parallel over batch: each core handles 8192 rows, gathers its
center rows from a replicated `cen` table via indirect DMA (the
embedding lookup), computes (data-center)^2, and DMAs per-partition
partials out; the host sums partials across partitions/cores.

The kernel is HBM-bound, and the 2e-2 rel-err budget dwarfs bf16
rounding noise (~3e-4 on this sum), so the host downcasts data/cen to
bf16 before upload — halving both the contiguous data stream and the
gather traffic.

Host prep: each core's 8192 rows are sorted by label so the gather's
descriptors walk the center table near-sequentially (DRAM locality);
the row sum is permutation-invariant. The first K_LIST[0] rows per
partition get their centers gathered on the host (cen0) so tile 0 is
two direct DMAs — compute starts ~5us before the first on-device
gather can land (labels DMA -> GPSIMD descriptor gen -> SWDGE).

Every tile gets its own SBUF buffer (64 rows/partition x 256 x bf16 x
2 tensors = 64KB/partition of the 208KB budget) so no DMA ever waits
on buffer recycling: all 9 data-tile DMAs issue the moment the NEFF
starts, and gathers issue as fast as GPSIMD generates descriptors.
Per tile, DVE computes the diff; the square+row-sum is split
ACT_FRAC/(1-ACT_FRAC) between ACT (Square w/ accumulate) and DVE
(affine_mul_reduce) so neither engine paces the stream.
"""

import os

import numpy as np

BATCH = 65536
DIM = 256
NUM_CLASSES = 100000
N_CORES = 8
B_CORE = BATCH // N_CORES  # 8192

P = 128               # SBUF partitions
R = B_CORE // P       # rows per partition (64)

# Tile 0 (host-gathered centers) first; small early tiles so the
# gather pipeline primes quickly.
_klist_env = os.environ.get("ACL_KLIST", "2,4,8,8,8,8,8,8,4,4,2")
K_LIST = [int(x) for x in _klist_env.split(",")]
assert sum(K_LIST) == R, K_LIST
NT = len(K_LIST)
DT = os.environ.get("ACL_DTYPE", "bf16")
ACT_FRAC = float(os.environ.get("ACL_ACT_FRAC", "0.69"))
TAIL_CHUNKS = int(os.environ.get("ACL_TAILCHUNKS", "1"))
FASTLAB = os.environ.get("ACL_FASTLAB", "0") == "1"
SORT = os.environ.get("ACL_SORT", "1") == "1"
HOSTG0 = os.environ.get("ACL_HOSTG0", "1") == "1"
IMPL = os.environ.get("ACL_IMPL", "tile")
DATADT = os.environ.get("ACL_DATADT", "same")  # f8 halves bytes but DVE sub drops to 1 elem/cycle
WARMG = os.environ.get("ACL_WARMG", "1") == "1"

_cached = {}


def _build_graph():
    from concourse import bass, bacc, mybir, tile

    nc = bacc.Bacc(
        "TRN2",
        target_bir_lowering=False,
        debug=False,
        num_devices=N_CORES,
    )
    f32 = mybir.dt.float32
    i32 = mybir.dt.int32
    vdt = mybir.dt.bfloat16 if DT == "bf16" else f32
    ddt = mybir.dt.float8e4 if DATADT == "f8" else vdt

    n_cols = 2 * (NT - 1 + TAIL_CHUNKS)  # (ACT, DVE) partial per chunk

    data_t = nc.dram_tensor("data", [P, R * DIM], ddt, kind="ExternalInput")
    lab_shape = [R, P] if FASTLAB else [P, R]
    lab_t = nc.dram_tensor("labels", lab_shape, i32, kind="ExternalInput")
    cen_t = nc.dram_tensor("cen", [NUM_CLASSES, DIM], vdt, kind="ExternalInput")
    if HOSTG0:
        cen0_t = nc.dram_tensor(
            "cen0", [P, K_LIST[0] * DIM], vdt, kind="ExternalInput"
        )
    out_t = nc.dram_tensor("out", [P, n_cols], f32, kind="ExternalOutput")

    with tile.TileContext(nc) as tc:
        with tc.tile_pool(name="persist", bufs=1) as persist:
            labs = persist.tile([P, R], i32)
            if FASTLAB:
                # Labels arrive transposed [64, 128]: 64 512B descriptors
                # instead of 128 256B ones; 8 DVE 32x32 block transposes
                # restore [128, 64], low-column blocks first.
                labs64 = persist.tile([R, P], i32)
                nc.sync.dma_start(out=labs64[:], in_=lab_t.ap()[:])
                for b in range(R // 32):
                    for a in range(P // 32):
                        nc.vector.transpose(
                            out=labs[32 * a:32 * a + 32, 32 * b:32 * b + 32],
                            in_=labs64[32 * b:32 * b + 32, 32 * a:32 * a + 32],
                        )
            else:
                nc.sync.dma_start(out=labs[:], in_=lab_t.ap()[:])

            # Dedicated buffers per tile: DMAs never wait on recycling.
            ctrs = [persist.tile([P, k * DIM], vdt, name=f"ctr{t}")
                    for t, k in enumerate(K_LIST)]
            dats = [persist.tile([P, k * DIM], ddt, name=f"dat{t}")
                    for t, k in enumerate(K_LIST)]
            parts = persist.tile([P, n_cols], f32)

            # All data-tile loads (and tile 0's direct center load) are
            # issued up front with no dependencies.
            off = 0
            for t, k in enumerate(K_LIST):
                nc.sync.dma_start(
                    out=dats[t][:],
                    in_=data_t.ap()[:, off * DIM:(off + k) * DIM],
                )
                off += k
            if HOSTG0:
                nc.sync.dma_start(out=ctrs[0][:], in_=cen0_t.ap()[:])

            if WARMG:
                # Dummy 2-row gather with constant offsets: pays the
                # SWDGE queue warmup latency before the real gathers
                # (which must wait for the labels DMA) need it.
                woff = persist.tile([P, 2], i32)
                wdst = persist.tile([P, 2 * DIM], vdt)
                nc.gpsimd.memset(woff[:], 0)
                nc.gpsimd.indirect_dma_start(
                    out=wdst[:],
                    out_offset=None,
                    in_=cen_t.ap()[:],
                    in_offset=bass.IndirectOffsetOnAxis(ap=woff[:], axis=0),
                )

            # Gathers: descriptor gen on GPSIMD (serial), paced only by
            # the labels DMA.
            off = 0
            for t, k in enumerate(K_LIST):
                if t > 0 or not HOSTG0:
                    nc.gpsimd.indirect_dma_start(
                        out=ctrs[t][:],
                        out_offset=None,
                        in_=cen_t.ap()[:],
                        in_offset=bass.IndirectOffsetOnAxis(
                            ap=labs[:, off:off + k], axis=0
                        ),
                    )
                off += k

            col = 0
            for t, k in enumerate(K_LIST):
                dat, ctr = dats[t], ctrs[t]
                last = t == len(K_LIST) - 1
                chunks = TAIL_CHUNKS if last and k % TAIL_CHUNKS == 0 else 1
                cw = k * DIM // chunks
                for c in range(chunks):
                    lo = c * cw
                    ca = lo + min(cw, max(32, int(cw * ACT_FRAC) // 32 * 32))
                    hi = lo + cw
                    nc.vector.tensor_tensor(
                        out=ctr[:, lo:hi], in0=dat[:, lo:hi], in1=ctr[:, lo:hi],
                        op=mybir.AluOpType.subtract,
                    )
                    # Squares' dead outputs overwrite the (consumed) data.
                    nc.scalar.activation(
                        dat[:, lo:ca], ctr[:, lo:ca],
                        mybir.ActivationFunctionType.Square,
                        accum_out=parts[:, col:col + 1],
                    )
                    if ca < hi:
                        nc.vector.affine_mul_reduce(
                            out=dat[:, ca:hi],
                            accum_out=parts[:, col + 1:col + 2],
                            in0=ctr[:, ca:hi], in1=ctr[:, ca:hi],
                            scale=1.0, bias=0.0,
                        )
                    else:
                        nc.vector.memset(parts[:, col + 1:col + 2], 0.0)
                    col += 2

            nc.sync.dma_start(out=out_t.ap()[:], in_=parts[:])

    nc.compile()
    return nc




def _build_graph_raw():
    """Raw-engine pipeline: same dataflow as the tile impl, but no
    TileContext prologue/epilogue barriers and explicit per-tile
    semaphores. Engine programs are in-order, so cumulative counting
    sems (sub/act/amr) are safe."""
    from contextlib import ExitStack

    from concourse import bass, bacc, mybir

    nc = bacc.Bacc(
        "TRN2",
        target_bir_lowering=False,
        debug=False,
        num_devices=N_CORES,
    )
    f32 = mybir.dt.float32
    i32 = mybir.dt.int32
    vdt = mybir.dt.bfloat16 if DT == "bf16" else f32
    ddt = mybir.dt.float8e4 if DATADT == "f8" else vdt

    chunks_of = [
        TAIL_CHUNKS if t == NT - 1 and k % TAIL_CHUNKS == 0 else 1
        for t, k in enumerate(K_LIST)
    ]
    n_chunks = sum(chunks_of)
    n_cols = 2 * n_chunks

    data_t = nc.dram_tensor("data", [P, R * DIM], ddt, kind="ExternalInput")
    lab_t = nc.dram_tensor("labels", [P, R], i32, kind="ExternalInput")
    cen_t = nc.dram_tensor("cen", [NUM_CLASSES, DIM], vdt, kind="ExternalInput")
    if HOSTG0:
        cen0_t = nc.dram_tensor(
            "cen0", [P, K_LIST[0] * DIM], vdt, kind="ExternalInput"
        )
    out_t = nc.dram_tensor("out", [P, n_cols], f32, kind="ExternalOutput")

    labs = nc.alloc_sbuf_tensor("labs", [P, R], i32)
    parts = nc.alloc_sbuf_tensor("parts", [P, n_cols], f32)
    bias = nc.alloc_sbuf_tensor("bias", [P, 1], f32)
    ctrs = [
        nc.alloc_sbuf_tensor(f"ctr{t}", [P, k * DIM], vdt)
        for t, k in enumerate(K_LIST)
    ]
    dats = [
        nc.alloc_sbuf_tensor(f"dat{t}", [P, k * DIM], ddt)
        for t, k in enumerate(K_LIST)
    ]
    if WARMG:
        woff = nc.alloc_sbuf_tensor("woff", [P, 2], i32)
        wdst = nc.alloc_sbuf_tensor("wdst", [P, 2 * DIM], vdt)

    with ExitStack() as es:
        block = es.enter_context(nc.Block(no_gpsimd_drain=True))
        lab_sem = es.enter_context(nc.semaphore("lab_sem"))
        out_sem = es.enter_context(nc.semaphore("out_sem"))
        sub_sem = es.enter_context(nc.semaphore("sub_sem"))
        act_sem = es.enter_context(nc.semaphore("act_sem"))
        amr_sem = es.enter_context(nc.semaphore("amr_sem"))
        warm_sem = es.enter_context(nc.semaphore("warm_sem"))
        wset_sem = es.enter_context(nc.semaphore("wset_sem"))
        dat_sems = [
            es.enter_context(nc.semaphore(f"dat_sem{t}")) for t in range(NT)
        ]
        ctr_sems = [
            es.enter_context(nc.semaphore(f"ctr_sem{t}")) for t in range(NT)
        ]

        def col_plan():
            col = 0
            for t, k in enumerate(K_LIST):
                cw = k * DIM // chunks_of[t]
                for c in range(chunks_of[t]):
                    lo = c * cw
                    ca = lo + min(cw, max(32, int(cw * ACT_FRAC) // 32 * 32))
                    yield t, lo, ca, lo + cw, col
                    col += 2

        @block.sync
        def _(sync: bass.BassEngine):
            sync.dma_start(out=labs.ap()[:], in_=lab_t.ap()[:]).then_inc(
                lab_sem, 16
            )
            if HOSTG0:
                sync.dma_start(out=ctrs[0].ap()[:], in_=cen0_t.ap()[:]).then_inc(
                    ctr_sems[0], 16
                )
            off = 0
            for t, k in enumerate(K_LIST):
                sync.dma_start(
                    out=dats[t].ap()[:],
                    in_=data_t.ap()[:, off * DIM:(off + k) * DIM],
                ).then_inc(dat_sems[t], 16)
                off += k
            sync.wait_ge(act_sem, n_chunks)
            sync.wait_ge(amr_sem, n_chunks)
            sync.dma_start(out=out_t.ap()[:], in_=parts.ap()[:]).then_inc(
                out_sem, 16
            )
            sync.wait_ge(out_sem, 16)

        @block.gpsimd
        def _(gpsimd: bass.BassEngine):
            if WARMG:
                gpsimd.memset(woff.ap()[:], 0).then_inc(wset_sem, 1)
                gpsimd.wait_ge(wset_sem, 1)
                gpsimd.indirect_dma_start(
                    out=wdst.ap()[:],
                    out_offset=None,
                    in_=cen_t.ap()[:],
                    in_offset=bass.IndirectOffsetOnAxis(ap=woff.ap()[:], axis=0),
                ).then_inc(warm_sem, 16)
            gpsimd.wait_ge(lab_sem, 16)
            off = 0
            for t, k in enumerate(K_LIST):
                if t > 0 or not HOSTG0:
                    gpsimd.indirect_dma_start(
                        out=ctrs[t].ap()[:],
                        out_offset=None,
                        in_=cen_t.ap()[:],
                        in_offset=bass.IndirectOffsetOnAxis(
                            ap=labs.ap()[:, off:off + k], axis=0
                        ),
                    ).then_inc(ctr_sems[t], 16)
                off += k

        @block.vector
        def _(vector: bass.BassEngine):
            vector.memset(bias.ap()[:], 0.0)
            seen = set()
            nsub = 0
            for t, lo, ca, hi, col in col_plan():
                if t not in seen:
                    seen.add(t)
                    vector.wait_ge(dat_sems[t], 16)
                    vector.wait_ge(ctr_sems[t], 16)
                vector.tensor_tensor(
                    out=ctrs[t].ap()[:, lo:hi],
                    in0=dats[t].ap()[:, lo:hi],
                    in1=ctrs[t].ap()[:, lo:hi],
                    op=mybir.AluOpType.subtract,
                ).then_inc(sub_sem, 1)
                nsub += 1
                if ca < hi:
                    vector.wait_ge(sub_sem, nsub)
                    vector.affine_mul_reduce(
                        out=dats[t].ap()[:, ca:hi],
                        accum_out=parts.ap()[:, col + 1:col + 2],
                        in0=ctrs[t].ap()[:, ca:hi],
                        in1=ctrs[t].ap()[:, ca:hi],
                        scale=1.0,
                        bias=0.0,
                    ).then_inc(amr_sem, 1)
                else:
                    vector.memset(parts.ap()[:, col + 1:col + 2], 0.0).then_inc(
                        amr_sem, 1
                    )

        @block.scalar
        def _(scalar: bass.BassEngine):
            i = 0
            for t, lo, ca, hi, col in col_plan():
                scalar.wait_ge(sub_sem, i + 1)
                scalar.activation(
                    dats[t].ap()[:, lo:ca],
                    ctrs[t].ap()[:, lo:ca],
                    mybir.ActivationFunctionType.Square,
                    bias=bias.ap()[:, :1],
                    accum_out=parts.ap()[:, col:col + 1],
                ).then_inc(act_sem, 1)
                i += 1

    nc.compile()
    return nc


def _get_graph():
    if "nc" not in _cached:
        _cached["nc"] = (
            _build_graph_raw() if IMPL == "raw" else _build_graph()
        )
    return _cached["nc"]


def _val_dtype():
    if DT == "bf16":
        import ml_dtypes

        return ml_dtypes.bfloat16
    return np.float32


def _data_dtype():
    if DATADT == "f8":
        import ml_dtypes

        return ml_dtypes.float8_e4m3
    return _val_dtype()


def _make_in_maps(data, cen, labels):
    vdt = _val_dtype()
    data = np.ascontiguousarray(np.asarray(data)).astype(_data_dtype())
    cen = np.ascontiguousarray(np.asarray(cen)).astype(vdt)
    labels = np.asarray(labels).astype(np.int32)
    in_maps = []
    for c in range(N_CORES):
        sl = slice(c * B_CORE, (c + 1) * B_CORE)
        dat_c = data[sl]
        lab_c = labels[sl]
        if SORT:
            # Sort rows by label: the gather descriptors then walk cen
            # near-sequentially (DRAM page locality). Sum is invariant.
            order = np.argsort(lab_c)
            dat_c = dat_c[order]
            lab_c = lab_c[order]
        lab2d = lab_c.reshape(P, R)
        if FASTLAB:
            lab_up = np.ascontiguousarray(lab2d.T)
        else:
            lab_up = np.ascontiguousarray(lab2d)
        m = {
            "data": dat_c.reshape(P, R * DIM),
            "labels": lab_up,
            "cen": cen,
        }
        if HOSTG0:
            k0 = K_LIST[0]
            m["cen0"] = cen[lab2d[:, :k0].ravel()].reshape(P, k0 * DIM)
        in_maps.append(m)
    return in_maps


def _run(data, cen, labels, trace=False):
    import time

    from concourse.bass_utils import run_bass_kernel_spmd

    nc = _get_graph()
    in_maps = _make_in_maps(data, cen, labels)
    last_err = None
    for attempt in range(4):
        try:
            res = run_bass_kernel_spmd(
                nc, in_maps, core_ids=list(range(N_CORES)), trace=trace
            )
        except Exception as e:  # transient NRT device flakes
            last_err = e
            time.sleep(2.0)
            continue
        total = float(
            np.sum(
                [res.results[i]["out"].astype(np.float64) for i in range(N_CORES)]
            )
        )
        if np.isfinite(total):  # rare cold-start flake: garbage gather
            return np.float32(total / BATCH), res
    if last_err is not None:
        raise last_err
    return np.float32(total / BATCH), res


def kernel(data, cen, labels):
    out, _ = _run(data, cen, labels)
    return out


# revision 16
# speedup vs baseline: 1.2822x; 1.0806x over previous
"""AdaptiveCenterLoss on 8 TRN2 NeuronCores.

loss = sum((data - cen[labels])**2) / BATCH

Data-parallel over batch: each core handles 8192 rows, gathers its
center rows from a replicated `cen` table via indirect DMA (the
embedding lookup), computes (data-center)^2, and DMAs per-partition
partials out; the host sums partials across partitions/cores.

The kernel is HBM-bound, and the 2e-2 rel-err budget dwarfs bf16
rounding noise (~3e-4 on this sum), so the host downcasts data/cen to
bf16 before upload — halving both the contiguous data stream and the
gather traffic.

Host prep: each core's 8192 rows are sorted by label so the gather's
descriptors walk the center table near-sequentially (DRAM locality);
the row sum is permutation-invariant. The first K_LIST[0] rows per
partition get their centers gathered on the host (cen0) so tile 0 is
two direct DMAs — compute starts ~5us before the first on-device
gather can land (labels DMA -> GPSIMD descriptor gen -> SWDGE).

Every tile gets its own SBUF buffer (64 rows/partition x 256 x bf16 x
2 tensors = 64KB/partition of the 208KB budget) so no DMA ever waits
on buffer recycling: all 9 data-tile DMAs issue the moment the NEFF
starts, and gathers issue as fast as GPSIMD generates descriptors.
Per tile, DVE computes the diff; the square+row-sum is split
ACT_FRAC/(1-ACT_FRAC) between ACT (Square w/ accumulate) and DVE
(affine_mul_reduce) so neither engine paces the stream.
"""

import os

import numpy as np

BATCH = 65536
DIM = 256
NUM_CLASSES = 100000
N_CORES = 8
B_CORE = BATCH // N_CORES  # 8192

P = 128               # SBUF partitions
R = B_CORE // P       # rows per partition (64)

# Tile 0 (host-gathered centers) first; small early tiles so the
# gather pipeline primes quickly.
_klist_env = os.environ.get("ACL_KLIST", "2,4,8,8,8,8,8,8,4,4,2")
K_LIST = [int(x) for x in _klist_env.split(",")]
assert sum(K_LIST) == R, K_LIST
NT = len(K_LIST)
DT = os.environ.get("ACL_DTYPE", "bf16")
ACT_FRAC = float(os.environ.get("ACL_ACT_FRAC", "0.69"))
TAIL_CHUNKS = int(os.environ.get("ACL_TAILCHUNKS", "1"))
FASTLAB = os.environ.get("ACL_FASTLAB", "0") == "1"
SORT = os.environ.get("ACL_SORT", "1") == "1"
HOSTG0 = os.environ.get("ACL_HOSTG0", "1") == "1"
IMPL = os.environ.get("ACL_IMPL", "tile")
DATADT = os.environ.get("ACL_DATADT", "same")  # f8 halves bytes but DVE sub drops to 1 elem/cycle
WARMG = os.environ.get("ACL_WARMG", "1") == "1"

_cached = {}


def _build_graph():
    from concourse import bass, bacc, mybir, tile

    nc = bacc.Bacc(
        "TRN2",
        target_bir_lowering=False,
        debug=False,
        num_devices=N_CORES,
    )
    f32 = mybir.dt.float32
    i32 = mybir.dt.int32
    vdt = {"bf16": mybir.dt.bfloat16, "f8": mybir.dt.float8e4}.get(DT, f32)
    ddt = mybir.dt.float8e4 if DATADT == "f8" else vdt

    n_cols = 2 * (NT - 1 + TAIL_CHUNKS)  # (ACT, DVE) partial per chunk

    data_t = nc.dram_tensor("data", [P, R * DIM], ddt, kind="ExternalInput")
    lab_shape = [R, P] if FASTLAB else [P, R]
    lab_t = nc.dram_tensor("labels", lab_shape, i32, kind="ExternalInput")
    cen_t = nc.dram_tensor("cen", [NUM_CLASSES, DIM], vdt, kind="ExternalInput")
    if HOSTG0:
        cen0_t = nc.dram_tensor(
            "cen0", [P, K_LIST[0] * DIM], vdt, kind="ExternalInput"
        )
    out_t = nc.dram_tensor("out", [P, n_cols], f32, kind="ExternalOutput")

    with tile.TileContext(nc) as tc:
        with tc.tile_pool(name="persist", bufs=1) as persist:
            labs = persist.tile([P, R], i32)
            if FASTLAB:
                # Labels arrive transposed [64, 128]: 64 512B descriptors
                # instead of 128 256B ones; 8 DVE 32x32 block transposes
                # restore [128, 64], low-column blocks first.
                labs64 = persist.tile([R, P], i32)
                nc.sync.dma_start(out=labs64[:], in_=lab_t.ap()[:])
                for b in range(R // 32):
                    for a in range(P // 32):
                        nc.vector.transpose(
                            out=labs[32 * a:32 * a + 32, 32 * b:32 * b + 32],
                            in_=labs64[32 * b:32 * b + 32, 32 * a:32 * a + 32],
                        )
            else:
                nc.sync.dma_start(out=labs[:], in_=lab_t.ap()[:])

            # Dedicated buffers per tile: DMAs never wait on recycling.
            ctrs = [persist.tile([P, k * DIM], vdt, name=f"ctr{t}")
                    for t, k in enumerate(K_LIST)]
            dats = [persist.tile([P, k * DIM], ddt, name=f"dat{t}")
                    for t, k in enumerate(K_LIST)]
            parts = persist.tile([P, n_cols], f32)

            # All data-tile loads (and tile 0's direct center load) are
            # issued up front with no dependencies.
            off = 0
            for t, k in enumerate(K_LIST):
                nc.sync.dma_start(
                    out=dats[t][:],
                    in_=data_t.ap()[:, off * DIM:(off + k) * DIM],
                )
                off += k
            if HOSTG0:
                nc.sync.dma_start(out=ctrs[0][:], in_=cen0_t.ap()[:])

            if WARMG:
                # Dummy 2-row gather with constant offsets: pays the
                # SWDGE queue warmup latency before the real gathers
                # (which must wait for the labels DMA) need it.
                woff = persist.tile([P, 2], i32)
                wdst = persist.tile([P, 2 * DIM], vdt)
                nc.gpsimd.memset(woff[:], 0)
                nc.gpsimd.indirect_dma_start(
                    out=wdst[:],
                    out_offset=None,
                    in_=cen_t.ap()[:],
                    in_offset=bass.IndirectOffsetOnAxis(ap=woff[:], axis=0),
                )

            # Gathers: descriptor gen on GPSIMD (serial), paced only by
            # the labels DMA.
            off = 0
            for t, k in enumerate(K_LIST):
                if t > 0 or not HOSTG0:
                    nc.gpsimd.indirect_dma_start(
                        out=ctrs[t][:],
                        out_offset=None,
                        in_=cen_t.ap()[:],
                        in_offset=bass.IndirectOffsetOnAxis(
                            ap=labs[:, off:off + k], axis=0
                        ),
                    )
                off += k

            col = 0
            for t, k in enumerate(K_LIST):
                dat, ctr = dats[t], ctrs[t]
                last = t == len(K_LIST) - 1
                chunks = TAIL_CHUNKS if last and k % TAIL_CHUNKS == 0 else 1
                cw = k * DIM // chunks
                for c in range(chunks):
                    lo = c * cw
                    ca = lo + min(cw, max(32, int(cw * ACT_FRAC) // 32 * 32))
                    hi = lo + cw
                    nc.vector.tensor_tensor(
                        out=ctr[:, lo:hi], in0=dat[:, lo:hi], in1=ctr[:, lo:hi],
                        op=mybir.AluOpType.subtract,
                    )
                    # Squares' dead outputs overwrite the (consumed) data.
                    nc.scalar.activation(
                        dat[:, lo:ca], ctr[:, lo:ca],
                        mybir.ActivationFunctionType.Square,
                        accum_out=parts[:, col:col + 1],
                    )
                    if ca < hi:
                        nc.vector.affine_mul_reduce(
                            out=dat[:, ca:hi],
                            accum_out=parts[:, col + 1:col + 2],
                            in0=ctr[:, ca:hi], in1=ctr[:, ca:hi],
                            scale=1.0, bias=0.0,
                        )
                    else:
                        nc.vector.memset(parts[:, col + 1:col + 2], 0.0)
                    col += 2

            nc.sync.dma_start(out=out_t.ap()[:], in_=parts[:])

    nc.compile()
    return nc




def _build_graph_raw():
    """Raw-engine pipeline: same dataflow as the tile impl, but no
    TileContext prologue/epilogue barriers and explicit per-tile
    semaphores. Engine programs are in-order, so cumulative counting
    sems (sub/act/amr) are safe."""
    from contextlib import ExitStack

    from concourse import bass, bacc, mybir

    nc = bacc.Bacc(
        "TRN2",
        target_bir_lowering=False,
        debug=False,
        num_devices=N_CORES,
    )
    f32 = mybir.dt.float32
    i32 = mybir.dt.int32
    vdt = {"bf16": mybir.dt.bfloat16, "f8": mybir.dt.float8e4}.get(DT, f32)
    ddt = mybir.dt.float8e4 if DATADT == "f8" else vdt

    chunks_of = [
        TAIL_CHUNKS if t == NT - 1 and k % TAIL_CHUNKS == 0 else 1
        for t, k in enumerate(K_LIST)
    ]
    n_chunks = sum(chunks_of)
    n_cols = 2 * n_chunks

    data_t = nc.dram_tensor("data", [P, R * DIM], ddt, kind="ExternalInput")
    lab_t = nc.dram_tensor("labels", [P, R], i32, kind="ExternalInput")
    cen_t = nc.dram_tensor("cen", [NUM_CLASSES, DIM], vdt, kind="ExternalInput")
    if HOSTG0:
        cen0_t = nc.dram_tensor(
            "cen0", [P, K_LIST[0] * DIM], vdt, kind="ExternalInput"
        )
    out_t = nc.dram_tensor("out", [P, n_cols], f32, kind="ExternalOutput")

    labs = nc.alloc_sbuf_tensor("labs", [P, R], i32)
    parts = nc.alloc_sbuf_tensor("parts", [P, n_cols], f32)
    bias = nc.alloc_sbuf_tensor("bias", [P, 1], f32)
    ctrs = [
        nc.alloc_sbuf_tensor(f"ctr{t}", [P, k * DIM], vdt)
        for t, k in enumerate(K_LIST)
    ]
    dats = [
        nc.alloc_sbuf_tensor(f"dat{t}", [P, k * DIM], ddt)
        for t, k in enumerate(K_LIST)
    ]
    if WARMG:
        woff = nc.alloc_sbuf_tensor("woff", [P, 2], i32)
        wdst = nc.alloc_sbuf_tensor("wdst", [P, 2 * DIM], vdt)

    with ExitStack() as es:
        block = es.enter_context(nc.Block(no_gpsimd_drain=True))
        lab_sem = es.enter_context(nc.semaphore("lab_sem"))
        out_sem = es.enter_context(nc.semaphore("out_sem"))
        sub_sem = es.enter_context(nc.semaphore("sub_sem"))
        act_sem = es.enter_context(nc.semaphore("act_sem"))
        amr_sem = es.enter_context(nc.semaphore("amr_sem"))
        warm_sem = es.enter_context(nc.semaphore("warm_sem"))
        wset_sem = es.enter_context(nc.semaphore("wset_sem"))
        dat_sems = [
            es.enter_context(nc.semaphore(f"dat_sem{t}")) for t in range(NT)
        ]
        ctr_sems = [
            es.enter_context(nc.semaphore(f"ctr_sem{t}")) for t in range(NT)
        ]

        def col_plan():
            col = 0
            for t, k in enumerate(K_LIST):
                cw = k * DIM // chunks_of[t]
                for c in range(chunks_of[t]):
                    lo = c * cw
                    ca = lo + min(cw, max(32, int(cw * ACT_FRAC) // 32 * 32))
                    yield t, lo, ca, lo + cw, col
                    col += 2

        @block.sync
        def _(sync: bass.BassEngine):
            sync.dma_start(out=labs.ap()[:], in_=lab_t.ap()[:]).then_inc(
                lab_sem, 16
            )
            if HOSTG0:
                sync.dma_start(out=ctrs[0].ap()[:], in_=cen0_t.ap()[:]).then_inc(
                    ctr_sems[0], 16
                )
            off = 0
            for t, k in enumerate(K_LIST):
                sync.dma_start(
                    out=dats[t].ap()[:],
                    in_=data_t.ap()[:, off * DIM:(off + k) * DIM],
                ).then_inc(dat_sems[t], 16)
                off += k
            sync.wait_ge(act_sem, n_chunks)
            sync.wait_ge(amr_sem, n_chunks)
            sync.dma_start(out=out_t.ap()[:], in_=parts.ap()[:]).then_inc(
                out_sem, 16
            )
            sync.wait_ge(out_sem, 16)

        @block.gpsimd
        def _(gpsimd: bass.BassEngine):
            if WARMG:
                gpsimd.memset(woff.ap()[:], 0).then_inc(wset_sem, 1)
                gpsimd.wait_ge(wset_sem, 1)
                gpsimd.indirect_dma_start(
                    out=wdst.ap()[:],
                    out_offset=None,
                    in_=cen_t.ap()[:],
                    in_offset=bass.IndirectOffsetOnAxis(ap=woff.ap()[:], axis=0),
                ).then_inc(warm_sem, 16)
            gpsimd.wait_ge(lab_sem, 16)
            off = 0
            for t, k in enumerate(K_LIST):
                if t > 0 or not HOSTG0:
                    gpsimd.indirect_dma_start(
                        out=ctrs[t].ap()[:],
                        out_offset=None,
                        in_=cen_t.ap()[:],
                        in_offset=bass.IndirectOffsetOnAxis(
                            ap=labs.ap()[:, off:off + k], axis=0
                        ),
                    ).then_inc(ctr_sems[t], 16)
                off += k

        @block.vector
        def _(vector: bass.BassEngine):
            vector.memset(bias.ap()[:], 0.0)
            seen = set()
            nsub = 0
            for t, lo, ca, hi, col in col_plan():
                if t not in seen:
                    seen.add(t)
                    vector.wait_ge(dat_sems[t], 16)
                    vector.wait_ge(ctr_sems[t], 16)
                vector.tensor_tensor(
                    out=ctrs[t].ap()[:, lo:hi],
                    in0=dats[t].ap()[:, lo:hi],
                    in1=ctrs[t].ap()[:, lo:hi],
                    op=mybir.AluOpType.subtract,
                ).then_inc(sub_sem, 1)
                nsub += 1
                if ca < hi:
                    vector.wait_ge(sub_sem, nsub)
                    vector.affine_mul_reduce(
                        out=dats[t].ap()[:, ca:hi],
                        accum_out=parts.ap()[:, col + 1:col + 2],
                        in0=ctrs[t].ap()[:, ca:hi],
                        in1=ctrs[t].ap()[:, ca:hi],
                        scale=1.0,
                        bias=0.0,
                    ).then_inc(amr_sem, 1)
                else:
                    vector.memset(parts.ap()[:, col + 1:col + 2], 0.0).then_inc(
                        amr_sem, 1
                    )

        @block.scalar
        def _(scalar: bass.BassEngine):
            i = 0
            for t, lo, ca, hi, col in col_plan():
                scalar.wait_ge(sub_sem, i + 1)
                scalar.activation(
                    dats[t].ap()[:, lo:ca],
                    ctrs[t].ap()[:, lo:ca],
                    mybir.ActivationFunctionType.Square,
                    bias=bias.ap()[:, :1],
                    accum_out=parts.ap()[:, col:col + 1],
                ).then_inc(act_sem, 1)
                i += 1

    nc.compile()
    return nc


def _get_graph():
    if "nc" not in _cached:
        _cached["nc"] = (
            _build_graph_raw() if IMPL == "raw" else _build_graph()
        )
    return _cached["nc"]


def _val_dtype():
    import ml_dtypes

    if DT == "bf16":
        return ml_dtypes.bfloat16
    if DT == "f8":
        return ml_dtypes.float8_e4m3
    return np.float32


def _data_dtype():
    if DATADT == "f8":
        import ml_dtypes

        return ml_dtypes.float8_e4m3
    return _val_dtype()


def _make_in_maps(data, cen, labels):
    vdt = _val_dtype()
    data = np.ascontiguousarray(np.asarray(data)).astype(_data_dtype())
    cen = np.ascontiguousarray(np.asarray(cen)).astype(vdt)
    labels = np.asarray(labels).astype(np.int32)
    in_maps = []
    for c in range(N_CORES):
        sl = slice(c * B_CORE, (c + 1) * B_CORE)
        dat_c = data[sl]
        lab_c = labels[sl]
        if SORT:
            # Sort rows by label: the gather descriptors then walk cen
            # near-sequentially (DRAM page locality). Sum is invariant.
            order = np.argsort(lab_c)
            dat_c = dat_c[order]
            lab_c = lab_c[order]
        lab2d = lab_c.reshape(P, R)
        if FASTLAB:
            lab_up = np.ascontiguousarray(lab2d.T)
        else:
            lab_up = np.ascontiguousarray(lab2d)
        m = {
            "data": dat_c.reshape(P, R * DIM),
            "labels": lab_up,
            "cen": cen,
        }
        if HOSTG0:
            k0 = K_LIST[0]
            m["cen0"] = cen[lab2d[:, :k0].ravel()].reshape(P, k0 * DIM)
        in_maps.append(m)
    return in_maps


def _run(data, cen, labels, trace=False):
    import time

    from concourse.bass_utils import run_bass_kernel_spmd

    nc = _get_graph()
    in_maps = _make_in_maps(data, cen, labels)
    last_err = None
    for attempt in range(4):
        try:
            res = run_bass_kernel_spmd(
                nc, in_maps, core_ids=list(range(N_CORES)), trace=trace
            )
        except Exception as e:  # transient NRT device flakes
            last_err = e
            time.sleep(2.0)
            continue
        total = float(
            np.sum(
                [res.results[i]["out"].astype(np.float64) for i in range(N_CORES)]
            )
        )
        if np.isfinite(total):  # rare cold-start flake: garbage gather
            return np.float32(total / BATCH), res
    if last_err is not None:
        raise last_err
    return np.float32(total / BATCH), res


def kernel(data, cen, labels):
    out, _ = _run(data, cen, labels)
    return out
